# revision 15
# baseline (speedup 1.0000x reference)
"""BorderLoss Trainium2 kernel (v4).

Reference (per element, then global mean over [64,512,512]):
    loss = softplus((1-2y)*x)   (stable BCE identity, y binary)
    m = (y > 0);  border = dilate3x3(m) - erode3x3(m)  (SAME, OOB ignored)
    w = 1 + border;  out = mean(loss * w)

Scheme (validated elementwise-exact vs reference in numpy):
  * v = 3x3 box-count of m with OOB=0, computed as horizontal 3-tap then
    vertical 3-tap.  Rows 0/511 get an extra 1.5x scale (folded into the
    tridiagonal matmul weights), after which ONE uniform band test
    |v - 4.5| <= 4.05  (i.e. 1 <= v <= 8) is exact everywhere except
    columns 0/511, fixed by a single strided STT with threshold 5.5
    (which also handles the corners exactly).
  * Horizontal 3-tap: outer pair (left+right) via one DVE bf16 2x add on
    a padded layout [P, 4, 516] (pads zero); the center tap is folded
    into the vertical matmul by running every tridiag/U/L pass twice,
    once on the outer-pair tensor and once on the center view.
  * Vertical 3-tap: per 128-row block, tridiagonal matmul on PE with
    single-entry U/L matrices carrying the cross-block rows.
  * loss: zh = (m - 0.5) * x  (DVE STT), then ACT Exp(scale=-2) and
    Ln(bias=1) = softplus((1-2m)x), with accum_out giving sum(l) free.
    A patched activation-table dict pins exp/ln/abs to the one table set
    containing all of them (baseline lost 18us to per-image reloads).
  * border-weighted sum: blocks 0-2 via ACT Abs(v-4.5) then one DVE bf16
    2x STT (<=4.05)*l; block 3 via two one-sided PSUM STTs (>=0.45 minus
    >=8.55)*l to balance ACT vs DVE load.  All reductions via accum_out
    into per-image [P,5] tiles, combined on host:
      total = sum(l) + [abs-path] + [>=0.45] - [>=8.55] - [colfix]
"""

import sys
import numpy as np

if "/opt/trn_rl_repo" not in sys.path:
    sys.path.insert(0, "/opt/trn_rl_repo")

# ---- pin exp/ln/abs/square to the single covering activation-table set ----
from concourse import hw_specs as _hw
import functools as _ft

if not getattr(_hw.get_activation_tables, "_borderloss_patched", False):
    _orig_tabs = _hw.get_activation_tables.__wrapped__

    @_ft.cache
    def _patched_tabs(module_arch):
        from concourse import mybir as _mb
        A = _mb.ActivationFunctionType
        strip = {A.Exp, A.Ln, A.Abs, A.Square}
        out = {}
        for k, v in _orig_tabs(module_arch).items():
            out[k] = v if k == "natural_log_exp_and_others" else v - strip
        return out

    _patched_tabs._borderloss_patched = True
    _hw.get_activation_tables = _patched_tabs

H = W = 512
P = 128
NB = 4               # 128-row blocks per image
FB = 516             # padded block width (data at cols 2..513, zeros at 1, 514)
FI = NB * FB         # 2064 padded free cols per image
FD = NB * W          # 2048 dense free cols per image
NACC = 5
N_CORES = 8
ABS_BLOCKS = 3       # blocks on the ACT-Abs path; the rest on one-sided STTs

_CACHE = {}


def _consts():
    import ml_dtypes
    bf = ml_dtypes.bfloat16
    tri = np.zeros((P, P), dtype=np.float64)
    for k in range(P):
        tri[k, max(0, k - 1):min(P, k + 2)] = 1.0
    t0 = tri.copy()
    t0[0:2, 0] = 1.5          # scale row 0 so the uniform band is exact
    t3 = tri.copy()
    t3[126:128, 127] = 1.5
    u = np.zeros((P, P), dtype=np.float64)
    u[0, 127] = 1.0           # next block's row 0 -> out row 127
    lm = np.zeros((P, P), dtype=np.float64)
    lm[127, 0] = 1.0          # prev block's row 127 -> out row 0
    wts = np.concatenate([t0, tri, t3, u, lm], axis=1).astype(bf)
    return wts


def _build(n_imgs):
    import concourse.bass as bass
    import concourse.bacc as bacc
    import concourse.tile as tile
    from concourse import mybir

    f32 = mybir.dt.float32
    bf16 = mybir.dt.bfloat16
    i32 = mybir.dt.int32
    Alu = mybir.AluOpType
    Act = mybir.ActivationFunctionType

    nc = bacc.Bacc(None, target_bir_lowering=False)
    x_d = nc.dram_tensor("x", [n_imgs, H, W], f32, kind="ExternalInput")
    y_d = nc.dram_tensor("y", [n_imgs, H, W], i32, kind="ExternalInput")
    w_d = nc.dram_tensor("wts", [P, 5 * P], bf16, kind="ExternalInput")
    acc_d = nc.dram_tensor("acc", [P, n_imgs * NACC], f32, kind="ExternalOutput")

    AB = ABS_BLOCKS
    FA = AB * W              # dense cols on the abs path
    with tile.TileContext(nc) as tc:
        with (
            tc.tile_pool(name="consts", bufs=1) as cpool,
            tc.tile_pool(name="inputs", bufs=1) as ipool,
            tc.tile_pool(name="work", bufs=4) as work,
            tc.tile_pool(name="accp", bufs=1) as apool,
            tc.tile_pool(name="ps", bufs=2, space=bass.MemorySpace.PSUM) as pp,
        ):
            wts = cpool.tile([P, 5 * P], bf16)
            nc.sync.dma_start(wts[:], w_d[:])
            bias_t = cpool.tile([P, 1], f32)
            nc.vector.memset(bias_t[:], -4.5)
            W_T0 = wts[:, 0:P]
            W_TRI = wts[:, P:2 * P]
            W_T3 = wts[:, 2 * P:3 * P]
            W_U = wts[:, 3 * P:4 * P]
            W_L = wts[:, 4 * P:5 * P]

            ms, xs, accs = [], [], []
            for i in range(n_imgs):
                m = ipool.tile([P, FI], bf16, tag=f"m{i}", name=f"m{i}")
                m3 = m.rearrange("p (b c) -> p b c", c=FB)
                # zero the pad columns (slots 1 and 514 of each block)
                nc.gpsimd.memset(m3[:, :, 1:FB - 1:FB - 3], 0)
                ms.append(m)
                xs.append(ipool.tile([P, FD], bf16, tag=f"x{i}", name=f"x{i}"))
                accs.append(apool.tile([P, NACC], f32, tag=f"a{i}", name=f"a{i}"))

            # laddered prefetch: keep ~2 images in flight so arrivals track
            # consumption order (all-at-once issue makes every transfer share
            # bandwidth and finish late together)
            tok = cpool.tile([P, 2 * n_imgs], bf16)
            for i in range(n_imgs):
                m3 = ms[i].rearrange("p (b c) -> p b c", c=FB)
                if i >= 1:
                    nc.gpsimd.tensor_copy(tok[:, 2 * i:2 * i + 1],
                                          ms[i - 1][:, 2:3])
                nc.gpsimd.dma_start(
                    m3[:, :, 2:FB - 2],
                    y_d[i].rearrange("(b p) w -> p b w", p=P))
                if i >= 1:
                    nc.gpsimd.tensor_copy(tok[:, 2 * i + 1:2 * i + 2],
                                          xs[i - 1][:, 0:1])
                else:
                    nc.gpsimd.tensor_copy(tok[:, 1:2], ms[0][:, 2:3])
                nc.gpsimd.dma_start(
                    xs[i].rearrange("p (b c) -> p b c", c=W),
                    x_d[i].rearrange("(b p) w -> p b w", p=P))

            # HAM warm-up: keep PE busy while the first loads land so real
            # matmuls run at 2.4 GHz from the start
            warm = pp.tile([P, FD], f32, tag="sp", name="warm")
            for _ in range(20):
                nc.tensor.matmul(warm[:, 0:W], wts[:, 0:P], wts[:, 0:4 * P],
                                 start=True, stop=True)

            def frontA(i):
                """t-add and the vertical matmuls (DVE t first, then PE)."""
                m, ac = ms[i], accs[i]
                m3 = m.rearrange("p (b c) -> p b c", c=FB)
                mc = m3[:, :, 2:FB - 2]

                t = work.tile([P, FI], bf16, tag="t", name=f"t{i}")
                nc.vector.tensor_add(t[:, 0:FI - 2], m[:, 0:FI - 2], m[:, 2:FI])
                t3 = t.rearrange("p (b c) -> p b c", c=FB)

                sp = pp.tile([P, FD], f32, tag="sp", name=f"sp{i}")

                def mm(b, wt, rhs, **kw):
                    nc.tensor.matmul(sp[:, b * W:(b + 1) * W], wt, rhs, **kw)

                for b, wt in ((0, W_T0), (1, W_TRI), (2, W_TRI)):
                    mm(b, wt, t3[:, b, 1:FB - 3], start=True, stop=False)
                    mm(b, wt, mc[:, b], start=False, stop=False)
                for b in (0, 1, 2):
                    mm(b, W_U, t3[:, b + 1, 1:FB - 3], start=False, stop=False)
                    mm(b, W_U, mc[:, b + 1], start=False, stop=(b == 0))
                for b in (1, 2):
                    mm(b, W_L, t3[:, b - 1, 1:FB - 3], start=False, stop=False)
                    mm(b, W_L, mc[:, b - 1], start=False, stop=True)
                mm(3, W_T3, t3[:, 3, 1:FB - 3], start=True, stop=False)
                mm(3, W_T3, mc[:, 3], start=False, stop=False)
                mm(3, W_L, t3[:, 2, 1:FB - 3], start=False, stop=False)
                mm(3, W_L, mc[:, 2], start=False, stop=True)
                return sp

            def frontB(i):
                """z-path on DVE, softplus (exp/ln) on ACT."""
                m, xb, ac = ms[i], xs[i], accs[i]
                m3 = m.rearrange("p (b c) -> p b c", c=FB)
                mc = m3[:, :, 2:FB - 2]

                m2 = work.tile([P, FD], bf16, tag="m2", name=f"m2{i}")
                nc.vector.tensor_scalar(
                    m2.rearrange("p (b c) -> p b c", c=W), mc, 0.5, None,
                    Alu.subtract)
                zh = work.tile([P, FD], bf16, tag="zh", name=f"zh{i}")
                nc.vector.tensor_mul(zh[:], m2[:], xb[:])
                eb = work.tile([P, FD], bf16, tag="eb", name=f"eb{i}")
                nc.scalar.activation(eb[:], zh[:], Act.Exp, scale=-2.0)
                lt = work.tile([P, FD], bf16, tag="lt", name=f"lt{i}")
                nc.scalar.activation(lt[:], eb[:], Act.Ln, bias=1.0,
                                     accum_out=ac[:, 0:1])
                return lt

            def back(i, sp, lt):
                """abs, border STTs, column fix, accumulator DMA-out."""
                ac = accs[i]
                lt3 = lt.rearrange("p (b c) -> p b c", c=W)

                ab = work.tile([P, FA], bf16, tag="ab", name=f"ab{i}")
                nc.scalar.activation(ab[:], sp[:, 0:FA], Act.Abs, bias=bias_t[:])
                u1 = work.tile([P, FA], bf16, tag="u1", name=f"u1{i}")
                nc.vector.scalar_tensor_tensor(
                    u1[:], ab[:], 4.05, lt[:, 0:FA], Alu.is_le, Alu.mult,
                    accum_out=ac[:, 1:2])

                u2 = work.tile([P, FD - FA], bf16, tag="u2", name=f"u2{i}")
                nc.vector.scalar_tensor_tensor(
                    u2[:], sp[:, FA:FD], 0.45, lt[:, FA:FD],
                    Alu.is_ge, Alu.mult, accum_out=ac[:, 2:3])
                u3 = work.tile([P, FD - FA], bf16, tag="u3", name=f"u3{i}")
                nc.vector.scalar_tensor_tensor(
                    u3[:], sp[:, FA:FD], 8.55, lt[:, FA:FD],
                    Alu.is_ge, Alu.mult, accum_out=ac[:, 3:4])

                sp3 = sp.rearrange("p (b c) -> p b c", c=W)
                ec = work.tile([P, 2 * NB], bf16, tag="ec", name=f"ec{i}")
                nc.vector.scalar_tensor_tensor(
                    ec.rearrange("p (b c) -> p b c", c=2),
                    sp3[:, :, ::W - 1], 5.5, lt3[:, :, ::W - 1],
                    Alu.is_ge, Alu.mult, accum_out=ac[:, 4:5])

                nc.sync.dma_start(acc_d[:, i * NACC:(i + 1) * NACC], ac[:])

            # software pipeline: DVE order per step is
            #   t_{i+1} | border-chain_i | m2/z_{i+1}
            # so the PSUM release chain never waits behind the next z-path
            sps, lts = {}, {}
            sps[0] = frontA(0)
            lts[0] = frontB(0)
            for i in range(1, n_imgs):
                sps[i] = frontA(i)
                back(i - 1, sps[i - 1], lts[i - 1])
                lts[i] = frontB(i)
            back(n_imgs - 1, sps[n_imgs - 1], lts[n_imgs - 1])

    nc.compile()
    return nc


def _get_nc(n_imgs):
    if n_imgs not in _CACHE:
        _CACHE[n_imgs] = _build(n_imgs)
    return _CACHE[n_imgs]


def _combine(acc, n_imgs):
    a = acc.reshape(P, n_imgs, NACC).astype(np.float64)
    return (a[:, :, 0].sum() + a[:, :, 1].sum() + a[:, :, 2].sum()
            - a[:, :, 3].sum() - a[:, :, 4].sum())


def kernel(x, y):
    from concourse import bass_utils

    n = x.shape[0]
    per = n // N_CORES
    nc = _get_nc(per)
    wts = _consts()
    x = np.ascontiguousarray(x, dtype=np.float32)
    y = np.ascontiguousarray(y, dtype=np.int32)
    in_maps = [
        {"x": x[c * per:(c + 1) * per], "y": y[c * per:(c + 1) * per],
         "wts": wts}
        for c in range(N_CORES)
    ]
    res = bass_utils.run_bass_kernel_spmd(nc, in_maps, core_ids=list(range(N_CORES)))
    total = 0.0
    for r in res.results:
        total += _combine(r["acc"], per)
    return np.float32(total / (n * H * W))


# revision 17
# speedup vs baseline: 1.0794x; 1.0794x over previous
"""BorderLoss Trainium2 kernel (v4).

Reference (per element, then global mean over [64,512,512]):
    loss = softplus((1-2y)*x)   (stable BCE identity, y binary)
    m = (y > 0);  border = dilate3x3(m) - erode3x3(m)  (SAME, OOB ignored)
    w = 1 + border;  out = mean(loss * w)

Scheme (validated elementwise-exact vs reference in numpy):
  * v = 3x3 box-count of m with OOB=0, computed as horizontal 3-tap then
    vertical 3-tap.  Rows 0/511 get an extra 1.5x scale (folded into the
    tridiagonal matmul weights), after which ONE uniform band test
    |v - 4.5| <= 4.05  (i.e. 1 <= v <= 8) is exact everywhere except
    columns 0/511, fixed by a single strided STT with threshold 5.5
    (which also handles the corners exactly).
  * Horizontal 3-tap: outer pair (left+right) via one DVE bf16 2x add on
    a padded layout [P, 4, 516] (pads zero); the center tap is folded
    into the vertical matmul by running every tridiag/U/L pass twice,
    once on the outer-pair tensor and once on the center view.
  * Vertical 3-tap: per 128-row block, tridiagonal matmul on PE with
    single-entry U/L matrices carrying the cross-block rows.
  * loss: zh = (m - 0.5) * x  (DVE STT), then ACT Exp(scale=-2) and
    Ln(bias=1) = softplus((1-2m)x), with accum_out giving sum(l) free.
    A patched activation-table dict pins exp/ln/abs to the one table set
    containing all of them (baseline lost 18us to per-image reloads).
  * border-weighted sum: blocks 0-2 via ACT Abs(v-4.5) then one DVE bf16
    2x STT (<=4.05)*l; block 3 via two one-sided PSUM STTs (>=0.45 minus
    >=8.55)*l to balance ACT vs DVE load.  All reductions via accum_out
    into per-image [P,5] tiles, combined on host:
      total = sum(l) + [abs-path] + [>=0.45] - [>=8.55] - [colfix]
"""

import sys
import numpy as np

if "/opt/trn_rl_repo" not in sys.path:
    sys.path.insert(0, "/opt/trn_rl_repo")

# ---- pin exp/ln/abs/square to the single covering activation-table set ----
from concourse import hw_specs as _hw
import functools as _ft

if not getattr(_hw.get_activation_tables, "_borderloss_patched", False):
    _orig_tabs = _hw.get_activation_tables.__wrapped__

    @_ft.cache
    def _patched_tabs(module_arch):
        from concourse import mybir as _mb
        A = _mb.ActivationFunctionType
        strip = {A.Exp, A.Ln, A.Abs, A.Square}
        out = {}
        for k, v in _orig_tabs(module_arch).items():
            out[k] = v if k == "natural_log_exp_and_others" else v - strip
        return out

    _patched_tabs._borderloss_patched = True
    _hw.get_activation_tables = _patched_tabs

H = W = 512
P = 128
NB = 4               # 128-row blocks per image
FB = 516             # padded block width (data at cols 2..513, zeros at 1, 514)
FI = NB * FB         # 2064 padded free cols per image
FD = NB * W          # 2048 dense free cols per image
NACC = 5
N_CORES = 8
ABS_BLOCKS = 3       # blocks on the ACT-Abs path; the rest on one-sided STTs

_CACHE = {}


def _consts():
    import ml_dtypes
    bf = ml_dtypes.bfloat16
    tri = np.zeros((P, P), dtype=np.float64)
    for k in range(P):
        tri[k, max(0, k - 1):min(P, k + 2)] = 1.0
    t0 = tri.copy()
    t0[0:2, 0] = 1.5          # scale row 0 so the uniform band is exact
    t3 = tri.copy()
    t3[126:128, 127] = 1.5
    u = np.zeros((P, P), dtype=np.float64)
    u[0, 127] = 1.0           # next block's row 0 -> out row 127
    lm = np.zeros((P, P), dtype=np.float64)
    lm[127, 0] = 1.0          # prev block's row 127 -> out row 0
    wts = np.concatenate([t0, tri, t3, u, lm], axis=1).astype(bf)
    return wts


def _build(n_imgs):
    import concourse.bass as bass
    import concourse.bacc as bacc
    import concourse.tile as tile
    from concourse import mybir

    f32 = mybir.dt.float32
    bf16 = mybir.dt.bfloat16
    i32 = mybir.dt.int32
    Alu = mybir.AluOpType
    Act = mybir.ActivationFunctionType

    nc = bacc.Bacc(None, target_bir_lowering=False)
    x_d = nc.dram_tensor("x", [n_imgs, H, W], f32, kind="ExternalInput")
    y_d = nc.dram_tensor("y", [n_imgs, H, W], i32, kind="ExternalInput")
    w_d = nc.dram_tensor("wts", [P, 5 * P], bf16, kind="ExternalInput")
    acc_d = nc.dram_tensor("acc", [P, n_imgs * NACC], f32, kind="ExternalOutput")

    AB = ABS_BLOCKS
    FA = AB * W              # dense cols on the abs path
    with tile.TileContext(nc) as tc:
        with (
            tc.tile_pool(name="consts", bufs=1) as cpool,
            tc.tile_pool(name="inputs", bufs=1) as ipool,
            tc.tile_pool(name="work", bufs=4) as work,
            tc.tile_pool(name="accp", bufs=1) as apool,
            tc.tile_pool(name="ps", bufs=2, space=bass.MemorySpace.PSUM) as pp,
        ):
            wts = cpool.tile([P, 5 * P], bf16)
            nc.sync.dma_start(wts[:], w_d[:])
            bias_t = cpool.tile([P, 1], f32)
            nc.vector.memset(bias_t[:], -4.5)
            W_T0 = wts[:, 0:P]
            W_TRI = wts[:, P:2 * P]
            W_T3 = wts[:, 2 * P:3 * P]
            W_U = wts[:, 3 * P:4 * P]
            W_L = wts[:, 4 * P:5 * P]

            ms, xs, accs = [], [], []
            for i in range(n_imgs):
                m = ipool.tile([P, FI], bf16, tag=f"m{i}", name=f"m{i}")
                m3 = m.rearrange("p (b c) -> p b c", c=FB)
                # zero the pad columns (slots 1 and 514 of each block)
                nc.gpsimd.memset(m3[:, :, 1:FB - 1:FB - 3], 0)
                ms.append(m)
                xs.append(ipool.tile([P, FD], bf16, tag=f"x{i}", name=f"x{i}"))
                accs.append(apool.tile([P, NACC], f32, tag=f"a{i}", name=f"a{i}"))

            # laddered prefetch: keep ~2 images in flight so arrivals track
            # consumption order (all-at-once issue makes every transfer share
            # bandwidth and finish late together)
            tok = cpool.tile([P, 2 * n_imgs], bf16)
            for i in range(n_imgs):
                m3 = ms[i].rearrange("p (b c) -> p b c", c=FB)
                if i >= 1:
                    nc.gpsimd.tensor_copy(tok[:, 2 * i:2 * i + 1],
                                          ms[i - 1][:, 2:3])
                nc.gpsimd.dma_start(
                    m3[:, :, 2:FB - 2],
                    y_d[i].rearrange("(b p) w -> p b w", p=P))
                if i >= 1:
                    nc.gpsimd.tensor_copy(tok[:, 2 * i + 1:2 * i + 2],
                                          xs[i - 1][:, 0:1])
                else:
                    nc.gpsimd.tensor_copy(tok[:, 1:2], ms[0][:, 2:3])
                nc.gpsimd.dma_start(
                    xs[i].rearrange("p (b c) -> p b c", c=W),
                    x_d[i].rearrange("(b p) w -> p b w", p=P))

            # HAM warm-up: keep PE busy while the first loads land so real
            # matmuls run at 2.4 GHz from the start
            warm = pp.tile([P, FD], f32, tag="sp", name="warm")
            for _ in range(20):
                nc.tensor.matmul(warm[:, 0:W], wts[:, 0:P], wts[:, 0:4 * P],
                                 start=True, stop=True)

            def frontA(i):
                """t-add and the vertical matmuls (DVE t first, then PE)."""
                m, ac = ms[i], accs[i]
                m3 = m.rearrange("p (b c) -> p b c", c=FB)
                mc = m3[:, :, 2:FB - 2]

                t = work.tile([P, FI], bf16, tag="t", name=f"t{i}")
                nc.vector.tensor_add(t[:, 0:FI - 2], m[:, 0:FI - 2], m[:, 2:FI])
                t3 = t.rearrange("p (b c) -> p b c", c=FB)

                sp = pp.tile([P, FD], f32, tag="sp", name=f"sp{i}")

                def mm(b, wt, rhs, **kw):
                    nc.tensor.matmul(sp[:, b * W:(b + 1) * W], wt, rhs, **kw)

                for b, wt in ((0, W_T0), (1, W_TRI), (2, W_TRI), (3, W_T3)):
                    mm(b, wt, t3[:, b, 1:FB - 3], start=True, stop=False)
                    mm(b, wt, mc[:, b], start=False, stop=False)
                for b in (0, 1, 2):
                    mm(b, W_U, t3[:, b + 1, 1:FB - 3], start=False, stop=False)
                    mm(b, W_U, mc[:, b + 1], start=False, stop=(b == 0))
                for b in (1, 2, 3):
                    mm(b, W_L, t3[:, b - 1, 1:FB - 3], start=False, stop=False)
                    mm(b, W_L, mc[:, b - 1], start=False, stop=(b != 0))
                return sp

            def frontB(i):
                """z-path on DVE, softplus (exp/ln) on ACT."""
                m, xb, ac = ms[i], xs[i], accs[i]
                m3 = m.rearrange("p (b c) -> p b c", c=FB)
                mc = m3[:, :, 2:FB - 2]

                m2 = work.tile([P, FD], bf16, tag="m2", name=f"m2{i}")
                nc.vector.tensor_scalar(
                    m2.rearrange("p (b c) -> p b c", c=W), mc, 0.5, None,
                    Alu.subtract)
                zh = work.tile([P, FD], bf16, tag="zh", name=f"zh{i}")
                nc.vector.tensor_mul(zh[:], m2[:], xb[:])
                eb = work.tile([P, FD], bf16, tag="eb", name=f"eb{i}")
                nc.scalar.activation(eb[:], zh[:], Act.Exp, scale=-2.0)
                lt = work.tile([P, FD], bf16, tag="lt", name=f"lt{i}")
                nc.scalar.activation(lt[:], eb[:], Act.Ln, bias=1.0,
                                     accum_out=ac[:, 0:1])
                return lt

            def back(i, sp, lt):
                """abs, border STTs, column fix, accumulator DMA-out."""
                ac = accs[i]
                lt3 = lt.rearrange("p (b c) -> p b c", c=W)

                ab = work.tile([P, FA], bf16, tag="ab", name=f"ab{i}")
                nc.scalar.activation(ab[:], sp[:, 0:FA], Act.Abs, bias=bias_t[:])
                u1 = work.tile([P, FA], bf16, tag="u1", name=f"u1{i}")
                nc.vector.scalar_tensor_tensor(
                    u1[:], ab[:], 4.05, lt[:, 0:FA], Alu.is_le, Alu.mult,
                    accum_out=ac[:, 1:2])

                u2 = work.tile([P, FD - FA], bf16, tag="u2", name=f"u2{i}")
                nc.vector.scalar_tensor_tensor(
                    u2[:], sp[:, FA:FD], 0.45, lt[:, FA:FD],
                    Alu.is_ge, Alu.mult, accum_out=ac[:, 2:3])
                u3 = work.tile([P, FD - FA], bf16, tag="u3", name=f"u3{i}")
                nc.vector.scalar_tensor_tensor(
                    u3[:], sp[:, FA:FD], 8.55, lt[:, FA:FD],
                    Alu.is_ge, Alu.mult, accum_out=ac[:, 3:4])

                sp3 = sp.rearrange("p (b c) -> p b c", c=W)
                ec = work.tile([P, 2 * NB], bf16, tag="ec", name=f"ec{i}")
                nc.vector.scalar_tensor_tensor(
                    ec.rearrange("p (b c) -> p b c", c=2),
                    sp3[:, :, ::W - 1], 5.5, lt3[:, :, ::W - 1],
                    Alu.is_ge, Alu.mult, accum_out=ac[:, 4:5])

                nc.sync.dma_start(acc_d[:, i * NACC:(i + 1) * NACC], ac[:])

            # software pipeline: DVE order per step is
            #   t_{i+1} | border-chain_i | m2/z_{i+1}
            # so the PSUM release chain never waits behind the next z-path
            sps, lts = {}, {}
            sps[0] = frontA(0)
            lts[0] = frontB(0)
            for i in range(1, n_imgs):
                sps[i] = frontA(i)
                back(i - 1, sps[i - 1], lts[i - 1])
                lts[i] = frontB(i)
            back(n_imgs - 1, sps[n_imgs - 1], lts[n_imgs - 1])

    nc.compile()
    return nc


def _get_nc(n_imgs):
    if n_imgs not in _CACHE:
        _CACHE[n_imgs] = _build(n_imgs)
    return _CACHE[n_imgs]


def _combine(acc, n_imgs):
    a = acc.reshape(P, n_imgs, NACC).astype(np.float64)
    return (a[:, :, 0].sum() + a[:, :, 1].sum() + a[:, :, 2].sum()
            - a[:, :, 3].sum() - a[:, :, 4].sum())


def kernel(x, y):
    from concourse import bass_utils

    n = x.shape[0]
    per = n // N_CORES
    nc = _get_nc(per)
    wts = _consts()
    x = np.ascontiguousarray(x, dtype=np.float32)
    y = np.ascontiguousarray(y, dtype=np.int32)
    in_maps = [
        {"x": x[c * per:(c + 1) * per], "y": y[c * per:(c + 1) * per],
         "wts": wts}
        for c in range(N_CORES)
    ]
    res = bass_utils.run_bass_kernel_spmd(nc, in_maps, core_ids=list(range(N_CORES)))
    total = 0.0
    for r in res.results:
        total += _combine(r["acc"], per)
    return np.float32(total / (n * H * W))


# revision 19
# speedup vs baseline: 1.1852x; 1.0980x over previous
"""BorderLoss Trainium2 kernel (v4).

Reference (per element, then global mean over [64,512,512]):
    loss = softplus((1-2y)*x)   (stable BCE identity, y binary)
    m = (y > 0);  border = dilate3x3(m) - erode3x3(m)  (SAME, OOB ignored)
    w = 1 + border;  out = mean(loss * w)

Scheme (validated elementwise-exact vs reference in numpy):
  * v = 3x3 box-count of m with OOB=0, computed as horizontal 3-tap then
    vertical 3-tap.  Rows 0/511 get an extra 1.5x scale (folded into the
    tridiagonal matmul weights), after which ONE uniform band test
    |v - 4.5| <= 4.05  (i.e. 1 <= v <= 8) is exact everywhere except
    columns 0/511, fixed by a single strided STT with threshold 5.5
    (which also handles the corners exactly).
  * Horizontal 3-tap: outer pair (left+right) via one DVE bf16 2x add on
    a padded layout [P, 4, 516] (pads zero); the center tap is folded
    into the vertical matmul by running every tridiag/U/L pass twice,
    once on the outer-pair tensor and once on the center view.
  * Vertical 3-tap: per 128-row block, tridiagonal matmul on PE with
    single-entry U/L matrices carrying the cross-block rows.
  * loss: zh = (m - 0.5) * x  (DVE STT), then ACT Exp(scale=-2) and
    Ln(bias=1) = softplus((1-2m)x), with accum_out giving sum(l) free.
    A patched activation-table dict pins exp/ln/abs to the one table set
    containing all of them (baseline lost 18us to per-image reloads).
  * border-weighted sum: blocks 0-2 via ACT Abs(v-4.5) then one DVE bf16
    2x STT (<=4.05)*l; block 3 via two one-sided PSUM STTs (>=0.45 minus
    >=8.55)*l to balance ACT vs DVE load.  All reductions via accum_out
    into per-image [P,5] tiles, combined on host:
      total = sum(l) + [abs-path] + [>=0.45] - [>=8.55] - [colfix]
"""

import sys
import numpy as np

if "/opt/trn_rl_repo" not in sys.path:
    sys.path.insert(0, "/opt/trn_rl_repo")

# ---- pin exp/ln/abs/square to the single covering activation-table set ----
from concourse import hw_specs as _hw
import functools as _ft

if not getattr(_hw.get_activation_tables, "_borderloss_patched", False):
    _orig_tabs = _hw.get_activation_tables.__wrapped__

    @_ft.cache
    def _patched_tabs(module_arch):
        from concourse import mybir as _mb
        A = _mb.ActivationFunctionType
        strip = {A.Exp, A.Ln, A.Abs, A.Square}
        out = {}
        for k, v in _orig_tabs(module_arch).items():
            out[k] = v if k == "natural_log_exp_and_others" else v - strip
        return out

    _patched_tabs._borderloss_patched = True
    _hw.get_activation_tables = _patched_tabs

H = W = 512
P = 128
NB = 4               # 128-row blocks per image
FB = 516             # padded block width (data at cols 2..513, zeros at 1, 514)
FI = NB * FB         # 2064 padded free cols per image
FD = NB * W          # 2048 dense free cols per image
NACC = 6
N_CORES = 8
ABS_BLOCKS = 3       # blocks on the ACT-Abs path; the rest on one-sided STTs

_CACHE = {}


def _consts():
    import ml_dtypes
    bf = ml_dtypes.bfloat16
    tri = np.zeros((P, P), dtype=np.float64)
    for k in range(P):
        tri[k, max(0, k - 1):min(P, k + 2)] = 1.0
    t0 = tri.copy()
    t0[0:2, 0] = 1.5          # scale row 0 so the uniform band is exact
    t3 = tri.copy()
    t3[126:128, 127] = 1.5
    u = np.zeros((P, P), dtype=np.float64)
    u[0, 127] = 1.0           # next block's row 0 -> out row 127
    lm = np.zeros((P, P), dtype=np.float64)
    lm[127, 0] = 1.0          # prev block's row 127 -> out row 0
    wts = np.concatenate([t0, tri, t3, u, lm], axis=1).astype(bf)
    return wts


def _build(n_imgs):
    import concourse.bass as bass
    import concourse.bacc as bacc
    import concourse.tile as tile
    from concourse import mybir

    f32 = mybir.dt.float32
    bf16 = mybir.dt.bfloat16
    i32 = mybir.dt.int32
    Alu = mybir.AluOpType
    Act = mybir.ActivationFunctionType

    nc = bacc.Bacc(None, target_bir_lowering=False)
    x_d = nc.dram_tensor("x", [n_imgs, H, W], f32, kind="ExternalInput")
    y_d = nc.dram_tensor("y", [n_imgs, H, W], i32, kind="ExternalInput")
    w_d = nc.dram_tensor("wts", [P, 5 * P], bf16, kind="ExternalInput")
    acc_d = nc.dram_tensor("acc", [P, n_imgs * NACC], f32, kind="ExternalOutput")

    AB = ABS_BLOCKS
    FA = AB * W              # dense cols on the abs path
    with tile.TileContext(nc) as tc:
        with (
            tc.tile_pool(name="consts", bufs=1) as cpool,
            tc.tile_pool(name="inputs", bufs=1) as ipool,
            tc.tile_pool(name="work", bufs=4) as work,
            tc.tile_pool(name="accp", bufs=1) as apool,
            tc.tile_pool(name="ps", bufs=2, space=bass.MemorySpace.PSUM) as pp,
        ):
            wts = cpool.tile([P, 5 * P], bf16)
            nc.sync.dma_start(wts[:], w_d[:])
            bias_t = cpool.tile([P, 1], f32)
            nc.vector.memset(bias_t[:], -4.5)
            W_T0 = wts[:, 0:P]
            W_TRI = wts[:, P:2 * P]
            W_T3 = wts[:, 2 * P:3 * P]
            W_U = wts[:, 3 * P:4 * P]
            W_L = wts[:, 4 * P:5 * P]

            ms, xs, accs = [], [], []
            for i in range(n_imgs):
                m = ipool.tile([P, FI], bf16, tag=f"m{i}", name=f"m{i}")
                m3 = m.rearrange("p (b c) -> p b c", c=FB)
                # zero the pad columns (slots 1 and 514 of each block)
                nc.gpsimd.memset(m3[:, :, 1:FB - 1:FB - 3], 0)
                ms.append(m)
                xs.append(ipool.tile([P, FD], bf16, tag=f"x{i}", name=f"x{i}"))
                accs.append(apool.tile([P, NACC], f32, tag=f"a{i}", name=f"a{i}"))

            # laddered prefetch: keep ~2 images in flight so arrivals track
            # consumption order (all-at-once issue makes every transfer share
            # bandwidth and finish late together)
            tok = cpool.tile([P, 2 * n_imgs], bf16)
            for i in range(n_imgs):
                m3 = ms[i].rearrange("p (b c) -> p b c", c=FB)
                if i >= 1:
                    nc.gpsimd.tensor_copy(tok[:, 2 * i:2 * i + 1],
                                          ms[i - 1][:, 2:3])
                nc.gpsimd.dma_start(
                    m3[:, :, 2:FB - 2],
                    y_d[i].rearrange("(b p) w -> p b w", p=P))
                if i >= 1:
                    nc.gpsimd.tensor_copy(tok[:, 2 * i + 1:2 * i + 2],
                                          xs[i - 1][:, 0:1])
                nc.gpsimd.dma_start(
                    xs[i].rearrange("p (b c) -> p b c", c=W),
                    x_d[i].rearrange("(b p) w -> p b w", p=P))

            # HAM warm-up: keep PE busy while the first loads land so real
            # matmuls run at 2.4 GHz from the start
            warm = pp.tile([P, FA], f32, tag="spA", name="warm")
            for _ in range(20):
                nc.tensor.matmul(warm[:, 0:W], wts[:, 0:P], wts[:, 0:4 * P],
                                 start=True, stop=True)

            def frontA(i):
                """t-add and the vertical matmuls (DVE t first, then PE)."""
                m, ac = ms[i], accs[i]
                m3 = m.rearrange("p (b c) -> p b c", c=FB)
                mc = m3[:, :, 2:FB - 2]

                t = work.tile([P, FI], bf16, tag="t", name=f"t{i}")
                nc.vector.tensor_add(t[:, 0:FI - 2], m[:, 0:FI - 2], m[:, 2:FI])
                t3 = t.rearrange("p (b c) -> p b c", c=FB)

                spA = pp.tile([P, FA], f32, tag="spA", name=f"spA{i}")
                spB = pp.tile([P, FD - FA], f32, tag="spB", name=f"spB{i}")

                def mm(b, wt, rhs, **kw):
                    if b < AB:
                        nc.tensor.matmul(spA[:, b * W:(b + 1) * W], wt, rhs, **kw)
                    else:
                        nc.tensor.matmul(spB[:, 0:W], wt, rhs, **kw)

                for b, wt in ((0, W_T0), (1, W_TRI), (2, W_TRI), (3, W_T3)):
                    mm(b, wt, t3[:, b, 1:FB - 3], start=True, stop=False)
                    mm(b, wt, mc[:, b], start=False, stop=False)
                for b in (0, 1, 2):
                    mm(b, W_U, t3[:, b + 1, 1:FB - 3], start=False, stop=False)
                    mm(b, W_U, mc[:, b + 1], start=False, stop=(b == 0))
                for b in (1, 2, 3):
                    mm(b, W_L, t3[:, b - 1, 1:FB - 3], start=False, stop=False)
                    mm(b, W_L, mc[:, b - 1], start=False, stop=(b != 0))
                return spA, spB

            def frontB(i):
                """z-path on DVE, softplus (exp/ln) on ACT."""
                m, xb, ac = ms[i], xs[i], accs[i]
                m3 = m.rearrange("p (b c) -> p b c", c=FB)
                mc = m3[:, :, 2:FB - 2]

                m2 = work.tile([P, FD], bf16, tag="m2", name=f"m2{i}")
                nc.vector.tensor_scalar(
                    m2.rearrange("p (b c) -> p b c", c=W), mc, 0.5, None,
                    Alu.subtract)
                zh = work.tile([P, FD], bf16, tag="zh", name=f"zh{i}")
                nc.vector.tensor_mul(zh[:], m2[:], xb[:])
                eb = work.tile([P, FD], bf16, tag="eb", name=f"eb{i}")
                nc.scalar.activation(eb[:], zh[:], Act.Exp, scale=-2.0)
                lt = work.tile([P, FD], bf16, tag="lt", name=f"lt{i}")
                nc.scalar.activation(lt[:], eb[:], Act.Ln, bias=1.0,
                                     accum_out=ac[:, 0:1])
                return lt

            def back(i, spp, lt):
                """abs, border STTs, column fix, accumulator DMA-out."""
                ac = accs[i]
                spA, spB = spp
                lt3 = lt.rearrange("p (b c) -> p b c", c=W)

                ab = work.tile([P, FA], bf16, tag="ab", name=f"ab{i}")
                nc.scalar.activation(ab[:], spA[:], Act.Abs, bias=bias_t[:])
                u1 = work.tile([P, FA], bf16, tag="u1", name=f"u1{i}")
                nc.vector.scalar_tensor_tensor(
                    u1[:], ab[:], 4.05, lt[:, 0:FA], Alu.is_le, Alu.mult,
                    accum_out=ac[:, 1:2])

                u2 = work.tile([P, FD - FA], bf16, tag="u2", name=f"u2{i}")
                nc.vector.scalar_tensor_tensor(
                    u2[:], spB[:], 0.45, lt[:, FA:FD],
                    Alu.is_ge, Alu.mult, accum_out=ac[:, 2:3])
                u3 = work.tile([P, FD - FA], bf16, tag="u3", name=f"u3{i}")
                nc.vector.scalar_tensor_tensor(
                    u3[:], spB[:], 8.55, lt[:, FA:FD],
                    Alu.is_ge, Alu.mult, accum_out=ac[:, 3:4])

                spA3 = spA.rearrange("p (b c) -> p b c", c=W)
                ec = work.tile([P, 2 * AB], bf16, tag="ec", name=f"ec{i}")
                nc.vector.scalar_tensor_tensor(
                    ec.rearrange("p (b c) -> p b c", c=2),
                    spA3[:, :, ::W - 1], 5.5, lt3[:, 0:AB, ::W - 1],
                    Alu.is_ge, Alu.mult, accum_out=ac[:, 4:5])
                ec2 = work.tile([P, 2], bf16, tag="ec2", name=f"ec2{i}")
                nc.vector.scalar_tensor_tensor(
                    ec2[:], spB[:, ::W - 1], 5.5, lt3[:, AB, ::W - 1],
                    Alu.is_ge, Alu.mult, accum_out=ac[:, 5:6])

                nc.sync.dma_start(acc_d[:, i * NACC:(i + 1) * NACC], ac[:])

            # software pipeline: DVE order per step is
            #   t_{i+1} | border-chain_i | m2/z_{i+1}
            # so the PSUM release chain never waits behind the next z-path
            sps, lts = {}, {}
            sps[0] = frontA(0)
            lts[0] = frontB(0)
            for i in range(1, n_imgs):
                sps[i] = frontA(i)
                back(i - 1, sps[i - 1], lts[i - 1])
                lts[i] = frontB(i)
            back(n_imgs - 1, sps[n_imgs - 1], lts[n_imgs - 1])

    nc.compile()
    return nc


def _get_nc(n_imgs):
    if n_imgs not in _CACHE:
        _CACHE[n_imgs] = _build(n_imgs)
    return _CACHE[n_imgs]


def _combine(acc, n_imgs):
    a = acc.reshape(P, n_imgs, NACC).astype(np.float64)
    return (a[:, :, 0].sum() + a[:, :, 1].sum() + a[:, :, 2].sum()
            - a[:, :, 3].sum() - a[:, :, 4].sum() - a[:, :, 5].sum())


def kernel(x, y):
    from concourse import bass_utils

    n = x.shape[0]
    per = n // N_CORES
    nc = _get_nc(per)
    wts = _consts()
    x = np.ascontiguousarray(x, dtype=np.float32)
    y = np.ascontiguousarray(y, dtype=np.int32)
    in_maps = [
        {"x": x[c * per:(c + 1) * per], "y": y[c * per:(c + 1) * per],
         "wts": wts}
        for c in range(N_CORES)
    ]
    res = bass_utils.run_bass_kernel_spmd(nc, in_maps, core_ids=list(range(N_CORES)))
    total = 0.0
    for r in res.results:
        total += _combine(r["acc"], per)
    return np.float32(total / (n * H * W))


# revision 20
# speedup vs baseline: 1.2155x; 1.0256x over previous
"""BorderLoss Trainium2 kernel (v4).

Reference (per element, then global mean over [64,512,512]):
    loss = softplus((1-2y)*x)   (stable BCE identity, y binary)
    m = (y > 0);  border = dilate3x3(m) - erode3x3(m)  (SAME, OOB ignored)
    w = 1 + border;  out = mean(loss * w)

Scheme (validated elementwise-exact vs reference in numpy):
  * v = 3x3 box-count of m with OOB=0, computed as horizontal 3-tap then
    vertical 3-tap.  Rows 0/511 get an extra 1.5x scale (folded into the
    tridiagonal matmul weights), after which ONE uniform band test
    |v - 4.5| <= 4.05  (i.e. 1 <= v <= 8) is exact everywhere except
    columns 0/511, fixed by a single strided STT with threshold 5.5
    (which also handles the corners exactly).
  * Horizontal 3-tap: outer pair (left+right) via one DVE bf16 2x add on
    a padded layout [P, 4, 516] (pads zero); the center tap is folded
    into the vertical matmul by running every tridiag/U/L pass twice,
    once on the outer-pair tensor and once on the center view.
  * Vertical 3-tap: per 128-row block, tridiagonal matmul on PE with
    single-entry U/L matrices carrying the cross-block rows.
  * loss: zh = (m - 0.5) * x  (DVE STT), then ACT Exp(scale=-2) and
    Ln(bias=1) = softplus((1-2m)x), with accum_out giving sum(l) free.
    A patched activation-table dict pins exp/ln/abs to the one table set
    containing all of them (baseline lost 18us to per-image reloads).
  * border-weighted sum: blocks 0-2 via ACT Abs(v-4.5) then one DVE bf16
    2x STT (<=4.05)*l; block 3 via two one-sided PSUM STTs (>=0.45 minus
    >=8.55)*l to balance ACT vs DVE load.  All reductions via accum_out
    into per-image [P,5] tiles, combined on host:
      total = sum(l) + [abs-path] + [>=0.45] - [>=8.55] - [colfix]
"""

import sys
import numpy as np

if "/opt/trn_rl_repo" not in sys.path:
    sys.path.insert(0, "/opt/trn_rl_repo")

# ---- pin exp/ln/abs/square to the single covering activation-table set ----
from concourse import hw_specs as _hw
import functools as _ft

if not getattr(_hw.get_activation_tables, "_borderloss_patched", False):
    _orig_tabs = _hw.get_activation_tables.__wrapped__

    @_ft.cache
    def _patched_tabs(module_arch):
        from concourse import mybir as _mb
        A = _mb.ActivationFunctionType
        strip = {A.Exp, A.Ln, A.Abs, A.Square}
        out = {}
        for k, v in _orig_tabs(module_arch).items():
            out[k] = v if k == "natural_log_exp_and_others" else v - strip
        return out

    _patched_tabs._borderloss_patched = True
    _hw.get_activation_tables = _patched_tabs

H = W = 512
P = 128
NB = 4               # 128-row blocks per image
FB = 516             # padded block width (data at cols 2..513, zeros at 1, 514)
FI = NB * FB         # 2064 padded free cols per image
FD = NB * W          # 2048 dense free cols per image
NACC = 6
N_CORES = 8
ABS_BLOCKS = 3       # blocks on the ACT-Abs path; the rest on one-sided STTs

_CACHE = {}


def _consts():
    import ml_dtypes
    bf = ml_dtypes.bfloat16
    tri = np.zeros((P, P), dtype=np.float64)
    for k in range(P):
        tri[k, max(0, k - 1):min(P, k + 2)] = 1.0
    t0 = tri.copy()
    t0[0:2, 0] = 1.5          # scale row 0 so the uniform band is exact
    t3 = tri.copy()
    t3[126:128, 127] = 1.5
    u = np.zeros((P, P), dtype=np.float64)
    u[0, 127] = 1.0           # next block's row 0 -> out row 127
    lm = np.zeros((P, P), dtype=np.float64)
    lm[127, 0] = 1.0          # prev block's row 127 -> out row 0
    wts = np.concatenate([t0, tri, t3, u, lm], axis=1).astype(bf)
    return wts


def _build(n_imgs):
    import concourse.bass as bass
    import concourse.bacc as bacc
    import concourse.tile as tile
    from concourse import mybir

    f32 = mybir.dt.float32
    bf16 = mybir.dt.bfloat16
    i32 = mybir.dt.int32
    Alu = mybir.AluOpType
    Act = mybir.ActivationFunctionType

    nc = bacc.Bacc(None, target_bir_lowering=False)
    x_d = nc.dram_tensor("x", [n_imgs, H, W], f32, kind="ExternalInput")
    y_d = nc.dram_tensor("y", [n_imgs, H, W], i32, kind="ExternalInput")
    w_d = nc.dram_tensor("wts", [P, 5 * P], bf16, kind="ExternalInput")
    acc_d = nc.dram_tensor("acc", [P, n_imgs * NACC], f32, kind="ExternalOutput")

    AB = ABS_BLOCKS
    FA = AB * W              # dense cols on the abs path
    with tile.TileContext(nc) as tc:
        with (
            tc.tile_pool(name="consts", bufs=1) as cpool,
            tc.tile_pool(name="inputs", bufs=1) as ipool,
            tc.tile_pool(name="work", bufs=4) as work,
            tc.tile_pool(name="accp", bufs=1) as apool,
            tc.tile_pool(name="ps", bufs=2, space=bass.MemorySpace.PSUM) as pp,
        ):
            wts = cpool.tile([P, 5 * P], bf16)
            nc.sync.dma_start(wts[:], w_d[:])
            bias_t = cpool.tile([P, 1], f32)
            nc.vector.memset(bias_t[:], -4.5)
            W_T0 = wts[:, 0:P]
            W_TRI = wts[:, P:2 * P]
            W_T3 = wts[:, 2 * P:3 * P]
            W_U = wts[:, 3 * P:4 * P]
            W_L = wts[:, 4 * P:5 * P]

            ms, xs, accs = [], [], []
            for i in range(n_imgs):
                m = ipool.tile([P, FI], bf16, tag=f"m{i}", name=f"m{i}")
                m3 = m.rearrange("p (b c) -> p b c", c=FB)
                # zero the pad columns (slots 1 and 514 of each block)
                nc.gpsimd.memset(m3[:, :, 1:FB - 1:FB - 3], 0)
                ms.append(m)
                xs.append(ipool.tile([P, FD], bf16, tag=f"x{i}", name=f"x{i}"))
                accs.append(apool.tile([P, NACC], f32, tag=f"a{i}", name=f"a{i}"))

            # laddered prefetch: keep ~2 images in flight so arrivals track
            # consumption order (all-at-once issue makes every transfer share
            # bandwidth and finish late together)
            tok = cpool.tile([P, 2 * n_imgs], bf16)
            for i in range(n_imgs):
                m3 = ms[i].rearrange("p (b c) -> p b c", c=FB)
                if i >= 1:
                    nc.gpsimd.tensor_copy(tok[:, 2 * i:2 * i + 1],
                                          ms[i - 1][:, 2:3])
                nc.gpsimd.dma_start(
                    m3[:, :, 2:FB - 2],
                    y_d[i].rearrange("(b p) w -> p b w", p=P))
                if i >= 1:
                    nc.gpsimd.tensor_copy(tok[:, 2 * i + 1:2 * i + 2],
                                          xs[i - 1][:, 0:1])
                nc.gpsimd.dma_start(
                    xs[i].rearrange("p (b c) -> p b c", c=W),
                    x_d[i].rearrange("(b p) w -> p b w", p=P))

            # HAM warm-up: keep PE busy while the first loads land so real
            # matmuls run at 2.4 GHz from the start
            warm = pp.tile([P, FA], f32, tag="spA", name="warm")
            for _ in range(20):
                nc.tensor.matmul(warm[:, 0:W], wts[:, 0:P], wts[:, 0:4 * P],
                                 start=True, stop=True)

            def frontA(i):
                """t-add and the vertical matmuls (DVE t first, then PE)."""
                m, ac = ms[i], accs[i]
                m3 = m.rearrange("p (b c) -> p b c", c=FB)
                mc = m3[:, :, 2:FB - 2]

                t = work.tile([P, FI], bf16, tag="t", name=f"t{i}")
                nc.vector.tensor_add(t[:, 0:FI - 2], m[:, 0:FI - 2], m[:, 2:FI])
                t3 = t.rearrange("p (b c) -> p b c", c=FB)

                spA = pp.tile([P, FA], f32, tag="spA", name=f"spA{i}")
                spB = pp.tile([P, FD - FA], f32, tag="spB", name=f"spB{i}")

                def mm(b, wt, rhs, **kw):
                    if b < AB:
                        nc.tensor.matmul(spA[:, b * W:(b + 1) * W], wt, rhs, **kw)
                    else:
                        nc.tensor.matmul(spB[:, 0:W], wt, rhs, **kw)

                for b, wt in ((0, W_T0), (1, W_TRI), (2, W_TRI), (3, W_T3)):
                    mm(b, wt, t3[:, b, 1:FB - 3], start=True, stop=False)
                    mm(b, wt, mc[:, b], start=False, stop=False)
                for b in (0, 1, 2):
                    mm(b, W_U, t3[:, b + 1, 1:FB - 3], start=False, stop=False)
                    mm(b, W_U, mc[:, b + 1], start=False, stop=(b == 0))
                for b in (1, 2, 3):
                    mm(b, W_L, t3[:, b - 1, 1:FB - 3], start=False, stop=False)
                    mm(b, W_L, mc[:, b - 1], start=False, stop=(b != 0))
                return spA, spB

            def frontB(i):
                """z-path on DVE, softplus (exp/ln) on ACT."""
                m, xb, ac = ms[i], xs[i], accs[i]
                m3 = m.rearrange("p (b c) -> p b c", c=FB)
                mc = m3[:, :, 2:FB - 2]

                m2 = work.tile([P, FD], bf16, tag="m2", name=f"m2{i}")
                nc.vector.tensor_scalar(
                    m2.rearrange("p (b c) -> p b c", c=W), mc, 0.5, None,
                    Alu.subtract)
                zh = work.tile([P, FD], bf16, tag="zh", name=f"zh{i}")
                nc.vector.tensor_mul(zh[:], m2[:], xb[:])
                eb = work.tile([P, FD], bf16, tag="eb", name=f"eb{i}")
                nc.scalar.activation(eb[:], zh[:], Act.Exp, scale=-2.0)
                lt = work.tile([P, FD], bf16, tag="lt", name=f"lt{i}")
                nc.scalar.activation(lt[:], eb[:], Act.Ln, bias=1.0,
                                     accum_out=ac[:, 0:1])
                return lt

            def back(i, spp, lt):
                """abs, border STTs, column fix, accumulator DMA-out."""
                ac = accs[i]
                spA, spB = spp
                lt3 = lt.rearrange("p (b c) -> p b c", c=W)

                ab = work.tile([P, FA], bf16, tag="ab", name=f"ab{i}")
                nc.scalar.activation(ab[:], spA[:], Act.Abs, bias=bias_t[:])
                spA3 = spA.rearrange("p (b c) -> p b c", c=W)
                ec = work.tile([P, 2 * AB], bf16, tag="ec", name=f"ec{i}")
                nc.vector.scalar_tensor_tensor(
                    ec.rearrange("p (b c) -> p b c", c=2),
                    spA3[:, :, ::W - 1], 5.5, lt3[:, 0:AB, ::W - 1],
                    Alu.is_ge, Alu.mult, accum_out=ac[:, 4:5])
                ec2 = work.tile([P, 2], bf16, tag="ec2", name=f"ec2{i}")
                nc.vector.scalar_tensor_tensor(
                    ec2[:], spB[:, ::W - 1], 5.5, lt3[:, AB, ::W - 1],
                    Alu.is_ge, Alu.mult, accum_out=ac[:, 5:6])

                u1 = work.tile([P, FA], bf16, tag="u1", name=f"u1{i}")
                nc.vector.scalar_tensor_tensor(
                    u1[:], ab[:], 4.05, lt[:, 0:FA], Alu.is_le, Alu.mult,
                    accum_out=ac[:, 1:2])

                u2 = work.tile([P, FD - FA], bf16, tag="u2", name=f"u2{i}")
                nc.vector.scalar_tensor_tensor(
                    u2[:], spB[:], 0.45, lt[:, FA:FD],
                    Alu.is_ge, Alu.mult, accum_out=ac[:, 2:3])
                u3 = work.tile([P, FD - FA], bf16, tag="u3", name=f"u3{i}")
                nc.vector.scalar_tensor_tensor(
                    u3[:], spB[:], 8.55, lt[:, FA:FD],
                    Alu.is_ge, Alu.mult, accum_out=ac[:, 3:4])

                nc.sync.dma_start(acc_d[:, i * NACC:(i + 1) * NACC], ac[:])

            # software pipeline: DVE order per step is
            #   t_{i+1} | border-chain_i | m2/z_{i+1}
            # so the PSUM release chain never waits behind the next z-path
            sps, lts = {}, {}
            sps[0] = frontA(0)
            lts[0] = frontB(0)
            for i in range(1, n_imgs):
                sps[i] = frontA(i)
                back(i - 1, sps[i - 1], lts[i - 1])
                lts[i] = frontB(i)
            back(n_imgs - 1, sps[n_imgs - 1], lts[n_imgs - 1])

    nc.compile()
    return nc


def _get_nc(n_imgs):
    if n_imgs not in _CACHE:
        _CACHE[n_imgs] = _build(n_imgs)
    return _CACHE[n_imgs]


def _combine(acc, n_imgs):
    a = acc.reshape(P, n_imgs, NACC).astype(np.float64)
    return (a[:, :, 0].sum() + a[:, :, 1].sum() + a[:, :, 2].sum()
            - a[:, :, 3].sum() - a[:, :, 4].sum() - a[:, :, 5].sum())


def kernel(x, y):
    from concourse import bass_utils

    n = x.shape[0]
    per = n // N_CORES
    nc = _get_nc(per)
    wts = _consts()
    x = np.ascontiguousarray(x, dtype=np.float32)
    y = np.ascontiguousarray(y, dtype=np.int32)
    in_maps = [
        {"x": x[c * per:(c + 1) * per], "y": y[c * per:(c + 1) * per],
         "wts": wts}
        for c in range(N_CORES)
    ]
    res = bass_utils.run_bass_kernel_spmd(nc, in_maps, core_ids=list(range(N_CORES)))
    total = 0.0
    for r in res.results:
        total += _combine(r["acc"], per)
    return np.float32(total / (n * H * W))
